# revision 2
# baseline (speedup 1.0000x reference)
"""GATv2 GNN (Graphormer-style) on 8 trn2 NeuronCores.

Strategy: edges sorted by destination and sharded by destination range
(6250 dsts/core). Per layer: each core computes its shard of xl = h@Wl and
xr = h@Wr, AllGathers xl (edge sources are global), then processes 49
dst-blocks of 128 nodes. Per 128-edge tile: indirect-DMA gather of xl[src],
one-hot matmuls broadcast xr[dst] / scatter-add exp-weighted messages into
PSUM. Softmax max-subtraction is skipped (logits are O(5); exact in fp32
since a per-dst constant cancels in the softmax ratio).
"""
import sys
import numpy as np

sys.path.insert(0, '/opt/trn_rl_repo')

N, E, IN_C, HID, HEADS, L, G, NCLS, MAXDEG = 50000, 800000, 128, 256, 4, 2, 64, 10, 10
HD = HID // HEADS
NCORES = 8
SH = N // NCORES            # 6250 real nodes per shard
NB = 49                     # dst blocks per core (49*128 = 6272 padded)
SHP = NB * 128
NEG = 0.2

_CACHE = {}


def _host_prep(edge_index, batch, deg, deg_emb, x):
    src = np.concatenate([edge_index[0], np.arange(N, dtype=np.int64)]).astype(np.int64)
    dst = np.concatenate([edge_index[1], np.arange(N, dtype=np.int64)]).astype(np.int64)
    order = np.argsort(dst, kind='stable')
    src, dst = src[order], dst[order]
    src_remap = ((src // SH) * SHP + (src % SH)).astype(np.int32)
    core_of = (dst // SH).astype(np.int32)
    loc = (dst % SH).astype(np.int32)
    lblk = loc // 128

    counts = np.zeros((NCORES, NB), dtype=np.int64)
    for c in range(NCORES):
        counts[c] = np.bincount(lblk[core_of == c], minlength=NB)
    KT = int(np.ceil(counts.max() / 128))

    idx_col = np.zeros((NCORES, NB, KT * 128, 2), dtype=np.int32)
    idx_col[:, :, :, 1] = 128  # dummy dst sentinel
    dl_row = np.full((NCORES, NB, KT * 128), 128, dtype=np.int32)
    for c in range(NCORES):
        m = core_of == c
        s_c, l_c, b_c = src_remap[m], loc[m], lblk[m]
        o2 = np.argsort(b_c, kind='stable')
        s_c, l_c, b_c = s_c[o2], l_c[o2], b_c[o2]
        starts = np.searchsorted(b_c, np.arange(NB))
        ends = np.searchsorted(b_c, np.arange(NB) + 1)
        for b in range(NB):
            k = ends[b] - starts[b]
            sl = slice(starts[b], ends[b])
            idx_col[c, b, :k, 0] = s_c[sl]
            idx_col[c, b, :k, 1] = l_c[sl] - b * 128
            dl_row[c, b, :k] = l_c[sl] - b * 128

    # resident idx layout [128, NB*2*KT]: per block, per partition p: KT (src,dl) pairs
    idx_res = idx_col.reshape(NCORES, NB, KT, 128, 2).transpose(0, 3, 1, 2, 4) \
                     .reshape(NCORES, 128, NB * KT * 2)
    # row layout for M build: [NB, KT*128] -> 1-partition rows per block
    dl_rows = dl_row.reshape(NCORES, NB, KT * 128)

    # per-shard inputs
    degg = deg_emb[np.clip(deg, 0, MAXDEG)].astype(np.float32)
    xT = np.zeros((NCORES, IN_C, SHP), dtype=np.float32)
    dege = np.zeros((NCORES, SHP, HID), dtype=np.float32)
    pool_oh = np.zeros((NCORES, NB, 128, G), dtype=np.float32)
    for c in range(NCORES):
        xs = x[c * SH:(c + 1) * SH]
        xT[c, :, :SH] = xs.T
        dege[c, :SH] = degg[c * SH:(c + 1) * SH]
        bs = batch[c * SH:(c + 1) * SH]
        oh = (bs[:, None] == np.arange(G)[None, :]).astype(np.float32)
        ohp = np.zeros((SHP, G), dtype=np.float32)
        ohp[:SH] = oh
        pool_oh[c] = ohp.reshape(NB, 128, G)
    cnt = np.bincount(batch.astype(np.int64), minlength=G).astype(np.float32)
    inv_cnt = (1.0 / np.maximum(cnt, 1.0)).reshape(G, 1).astype(np.float32)
    return KT, idx_res, dl_rows, xT, dege, pool_oh, inv_cnt


def _build(KT):
    from concourse import bass, mybir, tile, bacc
    from concourse.masks import make_identity
    F32 = mybir.dt.float32
    I32 = mybir.dt.int32
    AF = mybir.ActivationFunctionType
    OP = mybir.AluOpType

    nc = bacc.Bacc("TRN2", target_bir_lowering=False, debug=False,
                   enable_asserts=False, num_devices=NCORES)

    t_xT = nc.dram_tensor("xT", [IN_C, SHP], F32, kind="ExternalInput").ap()
    t_dege = nc.dram_tensor("dege", [SHP, HID], F32, kind="ExternalInput").ap()
    t_idx = nc.dram_tensor("idx", [128, NB * KT * 2], I32, kind="ExternalInput").ap()
    t_dlr = nc.dram_tensor("dlr", [NB, KT * 128], I32, kind="ExternalInput").ap()
    t_Win = nc.dram_tensor("Win", [IN_C, HID], F32, kind="ExternalInput").ap()
    t_Wl = nc.dram_tensor("Wl", [L, HID, HID], F32, kind="ExternalInput").ap()
    t_Wr = nc.dram_tensor("Wr", [L, HID, HID], F32, kind="ExternalInput").ap()
    t_att = nc.dram_tensor("attr", [L, 128, HID], F32, kind="ExternalInput").ap()
    t_gam = nc.dram_tensor("gamr", [L, 128, HID], F32, kind="ExternalInput").ap()
    t_bet = nc.dram_tensor("betr", [L, 128, HID], F32, kind="ExternalInput").ap()
    t_bcv = nc.dram_tensor("bcvr", [L, 128, HID], F32, kind="ExternalInput").ap()
    t_bin = nc.dram_tensor("binr", [128, HID], F32, kind="ExternalInput").ap()
    t_poh = nc.dram_tensor("poh", [NB, 128, G], F32, kind="ExternalInput").ap()
    t_icnt = nc.dram_tensor("icnt", [G, 1], F32, kind="ExternalInput").ap()
    t_Wc = nc.dram_tensor("Wc", [HID, NCLS], F32, kind="ExternalInput").ap()
    t_bc = nc.dram_tensor("bcr", [G, NCLS], F32, kind="ExternalInput").ap()
    t_out = nc.dram_tensor("out", [G, NCLS], F32, kind="ExternalOutput").ap()

    with tile.TileContext(nc) as tc:
        from contextlib import ExitStack
        with ExitStack() as ctx:
            cpool = ctx.enter_context(tc.tile_pool(name="const", bufs=1))
            dram = ctx.enter_context(tc.tile_pool(name="dram", bufs=1, space="DRAM"))
            sb = ctx.enter_context(tc.tile_pool(name="sb", bufs=3))
            sres = ctx.enter_context(tc.tile_pool(name="sres", bufs=1))
            ps = ctx.enter_context(tc.tile_pool(name="ps", bufs=2, space="PSUM"))
            pt = ctx.enter_context(tc.tile_pool(name="pt", bufs=1, space="PSUM"))
            pacc = ctx.enter_context(tc.tile_pool(name="pacc", bufs=2, space="PSUM"))
            ppool = ctx.enter_context(tc.tile_pool(name="ppool", bufs=1, space="PSUM"))

            ident = cpool.tile([128, 128], F32)
            make_identity(nc, ident[:])
            iota_i = cpool.tile([128, 128], I32)
            nc.gpsimd.iota(iota_i[:], pattern=[[1, 128]], base=0, channel_multiplier=0)
            iota_c = cpool.tile([128, 1], I32)
            nc.gpsimd.iota(iota_c[:], pattern=[[0, 1]], base=0, channel_multiplier=1)

            epsc = cpool.tile([128, 1], F32)
            nc.vector.memset(epsc[:], 1e-5)
            idx_res = sres.tile([128, NB * KT * 2], I32)
            nc.sync.dma_start(idx_res[:], t_idx)
            binr = cpool.tile([128, HID], F32)
            nc.sync.dma_start(binr[:], t_bin)

            h_a = dram.tile([SHP, HID], F32)
            h_b = dram.tile([SHP, HID], F32)
            xl_sh = dram.tile([SHP, HID], F32)
            xl_full = dram.tile([NCORES * SHP, HID], F32)

            # resident per-layer xr for own shard: [128, NB*HID]
            xr_res = sres.tile([128, NB * HID], F32)

            # ---- stage A: h0 = x @ Win + b_in + deg_emb[deg] ----
            winsb = cpool.tile([IN_C, HID], F32)
            nc.sync.dma_start(winsb[:], t_Win)
            for nt in range(NB):
                p0 = ps.tile([128, HID], F32, tag="mm", space="PSUM")
                nc.tensor.matmul(out=p0[:], lhsT=t_xT_sb(nc, sb, t_xT, nt),
                                 rhs=winsb[:], start=True, stop=True)
                dg = sb.tile([128, HID], F32, tag="dg")
                nc.sync.dma_start(dg[:], t_dege[nt * 128:(nt + 1) * 128, :])
                h0 = sb.tile([128, HID], F32, tag="h0")
                nc.vector.tensor_tensor(out=h0[:], in0=p0[:], in1=binr[:], op=OP.add)
                nc.vector.tensor_tensor(out=h0[:], in0=h0[:], in1=dg[:], op=OP.add)
                nc.sync.dma_start(h_a[nt * 128:(nt + 1) * 128, :], h0[:])

            layer_h = [(h_a, h_b), (h_b, h_a)]
            for l in range(L):
                h_cur, h_nxt = layer_h[l]
                attr = cpool.tile([128, HID], F32, tag=f"attr{l}")
                nc.sync.dma_start(attr[:], t_att[l])
                gamr = cpool.tile([128, HID], F32, tag=f"gamr{l}")
                nc.sync.dma_start(gamr[:], t_gam[l])
                betr = cpool.tile([128, HID], F32, tag=f"betr{l}")
                nc.sync.dma_start(betr[:], t_bet[l])
                bcvr = cpool.tile([128, HID], F32, tag=f"bcvr{l}")
                nc.sync.dma_start(bcvr[:], t_bcv[l])
                wl_sb = cpool.tile([128, 2 * HID], F32, tag=f"wl{l}")
                wr_sb = cpool.tile([128, 2 * HID], F32, tag=f"wr{l}")
                for half in range(2):
                    nc.sync.dma_start(wl_sb[:, half * HID:(half + 1) * HID],
                                      t_Wl[l, half * 128:(half + 1) * 128, :])
                    nc.sync.dma_start(wr_sb[:, half * HID:(half + 1) * HID],
                                      t_Wr[l, half * 128:(half + 1) * 128, :])

                # ---- B1: xl/xr for own shard ----
                for nt in range(NB):
                    hrow = sb.tile([128, HID], F32, tag="hrow")
                    nc.sync.dma_start(hrow[:], h_cur[nt * 128:(nt + 1) * 128, :])
                    hT = sb.tile([128, HID], F32, tag="hT")  # [f, n] halves
                    for half in range(2):
                        tp = pt.tile([128, 128], F32, tag="t128", space="PSUM")
                        nc.tensor.transpose(out=tp[:], in_=hrow[:, half * 128:(half + 1) * 128],
                                            identity=ident[:])
                        nc.scalar.activation(out=hT[:, half * 128:(half + 1) * 128],
                                             in_=tp[:], func=AF.Copy)
                    pxl = ps.tile([128, HID], F32, tag="mm", space="PSUM")
                    pxr = ps.tile([128, HID], F32, tag="mm2", space="PSUM")
                    for half in range(2):
                        nc.tensor.matmul(out=pxl[:], lhsT=hT[:, half * 128:(half + 1) * 128],
                                         rhs=wl_sb[:, half * HID:(half + 1) * HID],
                                         start=half == 0, stop=half == 1)
                        nc.tensor.matmul(out=pxr[:], lhsT=hT[:, half * 128:(half + 1) * 128],
                                         rhs=wr_sb[:, half * HID:(half + 1) * HID],
                                         start=half == 0, stop=half == 1)
                    xl_o = sb.tile([128, HID], F32, tag="xl_o")
                    nc.scalar.activation(out=xl_o[:], in_=pxl[:], func=AF.Copy)
                    nc.sync.dma_start(xl_sh[nt * 128:(nt + 1) * 128, :], xl_o[:])
                    nc.vector.tensor_copy(xr_res[:, nt * HID:(nt + 1) * HID], pxr[:])

                # ---- B2: AllGather xl ----
                nc.gpsimd.collective_compute(
                    "AllGather", OP.bypass, replica_groups=[list(range(NCORES))],
                    ins=[xl_sh[:].opt()], outs=[xl_full[:].opt()])

                # ---- B3: edge phase per dst block ----
                for b in range(NB):
                    acc = pacc.tile([128, HID + HEADS], F32, tag="acc", space="PSUM")
                    ioff = b * KT * 2
                    for t in range(KT):
                        src_col = idx_res[:, ioff + 2 * t:ioff + 2 * t + 1]
                        dl_col = idx_res[:, ioff + 2 * t + 1:ioff + 2 * t + 2]
                        xl_e = sb.tile([128, HID], F32, tag="xl_e")
                        nc.gpsimd.indirect_dma_start(
                            out=xl_e[:], out_offset=None, in_=xl_full[:],
                            in_offset=bass.IndirectOffsetOnAxis(ap=src_col, axis=0))
                        m2 = sb.tile([128, 128], F32, tag="m2")
                        nc.vector.tensor_tensor(out=m2[:], in0=dl_col.to_broadcast([128, 128]),
                                                in1=iota_i[:], op=OP.is_equal)
                        mt_ps = pt.tile([128, 128], F32, tag="t128", space="PSUM")
                        nc.tensor.transpose(out=mt_ps[:], in_=m2[:], identity=ident[:])
                        m_ = sb.tile([128, 128], F32, tag="m_")
                        nc.scalar.activation(out=m_[:], in_=mt_ps[:], func=AF.Copy)
                        pz = ps.tile([128, HID], F32, tag="mm", space="PSUM")
                        nc.tensor.matmul(out=pz[:], lhsT=m_[:],
                                         rhs=xr_res[:, b * HID:(b + 1) * HID],
                                         start=True, stop=False)
                        nc.tensor.matmul(out=pz[:], lhsT=ident[:], rhs=xl_e[:],
                                         start=False, stop=True)
                        zl = sb.tile([128, HID], F32, tag="zl")
                        rl = sb.tile([128, HID], F32, tag="rl")
                        nc.scalar.activation(out=rl[:], in_=pz[:], func=AF.Relu, scale=0.8)
                        nc.vector.tensor_scalar(out=zl[:], in0=pz[:], scalar1=NEG,
                                                scalar2=None, op0=OP.mult)
                        nc.vector.tensor_tensor(out=zl[:], in0=zl[:], in1=rl[:], op=OP.add)
                        wm = sb.tile([128, HID + HEADS], F32, tag="wm")
                        logit = sb.tile([128, HEADS], F32, tag="logit")
                        scr = sb.tile([128, HD], F32, tag="scr")
                        wat = sb.tile([128, HID], F32, tag="wat")
                        nc.vector.tensor_tensor(out=wat[:], in0=zl[:], in1=attr[:],
                                                op=OP.mult)
                        for hh in range(HEADS):
                            nc.scalar.activation(out=scr[:], in_=wat[:, hh * HD:(hh + 1) * HD],
                                                 func=AF.Copy,
                                                 accum_out=logit[:, hh:hh + 1])
                        nc.scalar.activation(out=wm[:, HID:HID + HEADS], in_=logit[:],
                                             func=AF.Exp)
                        for hh in range(HEADS):
                            nc.vector.tensor_scalar(
                                out=wm[:, hh * HD:(hh + 1) * HD],
                                in0=xl_e[:, hh * HD:(hh + 1) * HD],
                                scalar1=wm[:, HID + hh:HID + hh + 1], scalar2=None,
                                op0=OP.mult)
                        nc.tensor.matmul(out=acc[:], lhsT=m2[:], rhs=wm[:],
                                         start=t == 0, stop=t == KT - 1)
                    # finalize block
                    s_sb = sb.tile([128, HEADS], F32, tag="s_sb")
                    nc.scalar.activation(out=s_sb[:], in_=acc[:, HID:HID + HEADS],
                                         func=AF.Copy, bias=1e-16)
                    rcp = sb.tile([128, HEADS], F32, tag="rcp")
                    nc.vector.reciprocal(rcp[:], s_sb[:])
                    v = sb.tile([128, HID], F32, tag="v")
                    for hh in range(HEADS):
                        nc.vector.tensor_scalar(
                            out=v[:, hh * HD:(hh + 1) * HD],
                            in0=acc[:, hh * HD:(hh + 1) * HD],
                            scalar1=rcp[:, hh:hh + 1], scalar2=None, op0=OP.mult)
                    nc.vector.tensor_tensor(out=v[:], in0=v[:], in1=bcvr[:], op=OP.add)
                    nc.scalar.activation(out=v[:], in_=v[:], func=AF.Relu)
                    hrow2 = sb.tile([128, HID], F32, tag="hrow2")
                    nc.sync.dma_start(hrow2[:], h_cur[b * 128:(b + 1) * 128, :])
                    nc.vector.tensor_tensor(out=v[:], in0=v[:], in1=hrow2[:], op=OP.add)
                    # LayerNorm
                    msum = sb.tile([128, 1], F32, tag="msum")
                    scr2 = sb.tile([128, HID], F32, tag="scr2")
                    nc.scalar.activation(out=scr2[:], in_=v[:], func=AF.Copy,
                                         accum_out=msum[:])
                    mu = sb.tile([128, 1], F32, tag="mu")
                    nc.vector.tensor_scalar(out=mu[:], in0=msum[:], scalar1=1.0 / HID,
                                            scalar2=None, op0=OP.mult)
                    xc = sb.tile([128, HID], F32, tag="xc")
                    nc.vector.tensor_scalar(out=xc[:], in0=v[:], scalar1=mu[:, :1],
                                            scalar2=None, op0=OP.subtract)
                    ssq = sb.tile([128, 1], F32, tag="ssq")
                    xsq = sb.tile([128, HID], F32, tag="xsq")
                    nc.vector.tensor_tensor(out=xsq[:], in0=xc[:], in1=xc[:], op=OP.mult)
                    nc.scalar.activation(out=scr2[:], in_=xsq[:], func=AF.Copy,
                                         accum_out=ssq[:])
                    std = sb.tile([128, 1], F32, tag="std")
                    nc.scalar.activation(out=std[:], in_=ssq[:], func=AF.Sqrt,
                                         scale=1.0 / HID, bias=epsc[:, :1])
                    rstd = sb.tile([128, 1], F32, tag="rstd")
                    nc.vector.reciprocal(rstd[:], std[:])
                    hn = sb.tile([128, HID], F32, tag="hn")
                    nc.vector.tensor_scalar(out=hn[:], in0=xc[:], scalar1=rstd[:, :1],
                                            scalar2=None, op0=OP.mult)
                    nc.vector.tensor_tensor(out=hn[:], in0=hn[:], in1=gamr[:], op=OP.mult)
                    nc.vector.tensor_tensor(out=hn[:], in0=hn[:], in1=betr[:], op=OP.add)
                    nc.sync.dma_start(h_nxt[b * 128:(b + 1) * 128, :], hn[:])
                    if l == L - 1:
                        poh = sb.tile([128, G], F32, tag="poh")
                        nc.sync.dma_start(poh[:], t_poh[b])
                        if b == 0:
                            ppool_t = ppool.tile([G, HID], F32, tag="ppool", space="PSUM")
                            _build.ppool_t = ppool_t
                        nc.tensor.matmul(out=_build.ppool_t[:], lhsT=poh[:], rhs=hn[:],
                                         start=b == 0, stop=b == NB - 1)

            # ---- stage C: pool + classifier ----
            pool_sb = sb.tile([G, HID], F32, tag="pool_sb")
            nc.scalar.activation(out=pool_sb[:], in_=_build.ppool_t[:], func=AF.Copy)
            pl_in = dram.tile([G, HID], F32)
            pl_out = dram.tile([G, HID], F32)
            nc.sync.dma_start(pl_in[:], pool_sb[:])
            nc.gpsimd.collective_compute(
                "AllReduce", OP.add, replica_groups=[list(range(NCORES))],
                ins=[pl_in[:].opt()], outs=[pl_out[:].opt()])
            pooled = sb.tile([G, HID], F32, tag="pooled")
            nc.sync.dma_start(pooled[:], pl_out[:])
            icnt = sb.tile([G, 1], F32, tag="icnt")
            nc.sync.dma_start(icnt[:], t_icnt)
            nc.vector.tensor_scalar(out=pooled[:], in0=pooled[:], scalar1=icnt[:, :1],
                                    scalar2=None, op0=OP.mult)
            wc_sb = sb.tile([128, 2 * NCLS], F32, tag="wc")
            for half in range(2):
                nc.sync.dma_start(wc_sb[:, half * NCLS:(half + 1) * NCLS],
                                  t_Wc[half * 128:(half + 1) * 128, :])
            pcls = ppool.tile([G, NCLS], F32, tag="ppool", space="PSUM")
            for half in range(2):
                tp = pt.tile([128, G], F32, tag="t128", space="PSUM")
                nc.tensor.transpose(out=tp[:], in_=pooled[:, half * 128:(half + 1) * 128],
                                    identity=ident[:G, :G])
                ptc = sb.tile([128, G], F32, tag="ptc")
                nc.scalar.activation(out=ptc[:], in_=tp[:], func=AF.Copy)
                nc.tensor.matmul(out=pcls[:], lhsT=ptc[:],
                                 rhs=wc_sb[:, half * NCLS:(half + 1) * NCLS],
                                 start=half == 0, stop=half == 1)
            bc_sb = sb.tile([G, NCLS], F32, tag="bc")
            nc.sync.dma_start(bc_sb[:], t_bc)
            res = sb.tile([G, NCLS], F32, tag="resout")
            nc.vector.tensor_tensor(out=res[:], in0=pcls[:], in1=bc_sb[:], op=OP.add)
            nc.sync.dma_start(t_out, res[:])

    nc.compile()
    return nc


def t_xT_sb(nc, sb, t_xT, nt):
    from concourse import mybir
    F32 = mybir.dt.float32
    tile_ = sb.tile([IN_C, 128], F32, tag="xTt")
    nc.sync.dma_start(tile_[:], t_xT[:, nt * 128:(nt + 1) * 128])
    return tile_[:]


def kernel(**inputs):
    from concourse import bass_utils
    x = np.asarray(inputs["x"], dtype=np.float32)
    edge_index = np.asarray(inputs["edge_index"])
    batch = np.asarray(inputs["batch"])
    deg = np.asarray(inputs["deg"])
    Win = np.asarray(inputs["Win"], dtype=np.float32)
    b_in = np.asarray(inputs["b_in"], dtype=np.float32)
    deg_emb = np.asarray(inputs["deg_emb"], dtype=np.float32)
    Wl = np.asarray(inputs["Wl"], dtype=np.float32)
    Wr = np.asarray(inputs["Wr"], dtype=np.float32)
    att = np.asarray(inputs["att"], dtype=np.float32)
    b_conv = np.asarray(inputs["b_conv"], dtype=np.float32)
    gamma = np.asarray(inputs["gamma"], dtype=np.float32)
    beta = np.asarray(inputs["beta"], dtype=np.float32)
    Wc = np.asarray(inputs["Wc"], dtype=np.float32)
    b_c = np.asarray(inputs["b_c"], dtype=np.float32)

    KT, idx_res, dl_rows, xT, dege, pool_oh, inv_cnt = _host_prep(
        edge_index, batch, deg, deg_emb, x)

    if KT not in _CACHE:
        _CACHE[KT] = _build(KT)
    nc = _CACHE[KT]

    att_rep = np.repeat(att.reshape(L, 1, HID), 128, axis=1).astype(np.float32)
    gam_rep = np.repeat(gamma.reshape(L, 1, HID), 128, axis=1).astype(np.float32)
    bet_rep = np.repeat(beta.reshape(L, 1, HID), 128, axis=1).astype(np.float32)
    bcv_rep = np.repeat(b_conv.reshape(L, 1, HID), 128, axis=1).astype(np.float32)
    bin_rep = np.repeat(b_in.reshape(1, HID), 128, axis=0).astype(np.float32)
    bc_rep = np.repeat(b_c.reshape(1, NCLS), G, axis=0).astype(np.float32)

    in_maps = []
    for c in range(NCORES):
        in_maps.append({
            "xT": xT[c], "dege": dege[c], "idx": idx_res[c], "dlr": dl_rows[c],
            "Win": Win, "Wl": Wl, "Wr": Wr, "attr": att_rep, "gamr": gam_rep,
            "betr": bet_rep, "bcvr": bcv_rep, "binr": bin_rep,
            "poh": pool_oh[c], "icnt": inv_cnt, "Wc": Wc, "bcr": bc_rep,
        })
    res = bass_utils.run_bass_kernel_spmd(nc, in_maps, core_ids=list(range(NCORES)))
    kernel.last_results = res
    return res.results[0]["out"].astype(np.float32)



# revision 10
# speedup vs baseline: 2.9294x; 2.9294x over previous
"""GATv2 GNN (Graphormer-style) on 8 trn2 NeuronCores.

Strategy: edges sorted by destination, sharded by destination range
(6250 dsts/core -> 49 blocks of 128 dsts). Per layer: each core computes
its shard of xl_aug=[h@Wl | 0.6*att.xl] (384-col padded rows) and xr,
AllGathers xl_aug (bf16), then per dst block gathers all its edges'
xl_aug[src] rows with TWO dma_gather instructions (int16 indices limit
one gather to 25088 rows, so the node table is split in halves) plus a
precomputed one-hot mask load. Per 128-edge tile attention is pure
matmuls: leaky_relu(p) = 0.6*p + 0.4*|p| so
  logit = 0.6*att.xl[src] + att04 . |pz|   (the xr-side linear term is
constant per destination and cancels in the softmax). pz^T is formed by
mask matmuls + transpose-accumulate; messages scatter-add into PSUM via
the one-hot mask matmul. All matmul data bf16; accumulation fp32.
"""
import sys
import numpy as np
import ml_dtypes

sys.path.insert(0, '/opt/trn_rl_repo')

BF = ml_dtypes.bfloat16

N, E, IN_C, HID, HEADS, L, G, NCLS, MAXDEG = 50000, 800000, 128, 256, 4, 2, 64, 10, 10
HD = HID // HEADS
NCORES = 8
SH = N // NCORES            # 6250 real nodes per shard
NB = 49                     # dst blocks per core (49*128 = 6272 padded)
SHP = NB * 128
AUG = HID + HEADS           # 260 useful cols
ROW = 384                   # padded gather row (768B, 256B-multiple)
HALFR = NCORES * SHP // 2   # 25088 rows per gather table half (int16 range)

_CACHE = {}


def _host_prep(edge_index, batch, deg, deg_emb, x, b_in):
    src = np.concatenate([edge_index[0], np.arange(N, dtype=np.int64)]).astype(np.int64)
    dst = np.concatenate([edge_index[1], np.arange(N, dtype=np.int64)]).astype(np.int64)
    order = np.argsort(dst, kind='stable')
    src, dst = src[order], dst[order]
    src_remap = ((src // SH) * SHP + (src % SH)).astype(np.int64)
    core_of = (dst // SH).astype(np.int64)
    loc = (dst % SH).astype(np.int64)
    lblk = loc // 128

    # collect per (core, block, half) edge lists
    ed = [[[None, None] for _ in range(NB)] for _ in range(NCORES)]
    cnts = np.zeros((NCORES, NB, 2), dtype=np.int64)
    for c in range(NCORES):
        m = core_of == c
        s_c, l_c, b_c = src_remap[m], loc[m], lblk[m]
        o2 = np.argsort(b_c, kind='stable')
        s_c, l_c, b_c = s_c[o2], l_c[o2], b_c[o2]
        starts = np.searchsorted(b_c, np.arange(NB))
        ends = np.searchsorted(b_c, np.arange(NB) + 1)
        for b in range(NB):
            sl = slice(starts[b], ends[b])
            ss = s_c[sl]
            dl = l_c[sl] - b * 128
            if b == NB - 1:
                # fake self-edges for the 22 padding dsts: keeps softmax
                # denominator > 0 so LayerNorm of padding rows stays finite
                pads = np.arange(SH % 128, 128, dtype=np.int64)
                ss = np.concatenate([ss, np.zeros(len(pads), np.int64)])
                dl = np.concatenate([dl, pads])
            hm = ss >= HALFR
            for h in range(2):
                sel = hm if h else ~hm
                ed[c][b][h] = (ss[sel] - h * HALFR, dl[sel])
                cnts[c, b, h] = sel.sum()

    slots = np.maximum(0, np.ceil(cnts.max(axis=0) / 128)).astype(np.int64)  # [NB,2]
    ktb = (slots[:, 0] + slots[:, 1]).astype(np.int64)
    T = int(ktb.sum())
    toff = np.zeros(NB + 1, dtype=np.int64)
    toff[1:] = np.cumsum(ktb)
    IT = T * 8   # int16 idx cols: per slot-tile 128 idxs -> 8 cols of 16 rows

    idx16 = np.zeros((NCORES, 16, IT), dtype=np.int16)
    masks = np.zeros((NCORES, 128, T * 256), dtype=np.uint16)
    ONE = np.float32(1.0).astype(BF).view(np.uint16)
    for c in range(NCORES):
        for b in range(NB):
            for h in range(2):
                sl = int(slots[b, h])
                if sl == 0:
                    continue
                t0 = toff[b] + (slots[b, 0] if h else 0)
                ss, dl = ed[c][b][h]
                n = len(ss)
                L128 = sl * 128
                idpad = np.zeros(L128, dtype=np.int16)
                idpad[:n] = ss.astype(np.int16)
                # wrapped layout: idx j -> [j%16, j//16]
                idx16[c, :, t0 * 8:(t0 + sl) * 8] = idpad.reshape(-1, 16).T
                j = np.arange(n)
                tt = t0 + j // 128
                e = j % 128
                masks[c, e, tt * 256 + dl] = ONE            # m2[e, d]
                masks[c, dl, tt * 256 + 128 + e] = ONE      # m_[d, e]

    idx_res = np.tile(idx16, (1, 8, 1))  # replicate into 128 partitions

    degg = (deg_emb[np.clip(deg, 0, MAXDEG)] + b_in[None, :]).astype(np.float32)
    xT = np.zeros((NCORES, IN_C, SHP), dtype=BF)
    dege = np.zeros((NCORES, SHP, HID), dtype=BF)
    pool_oh = np.zeros((NCORES, NB, 128, G), dtype=BF)
    for c in range(NCORES):
        xs = x[c * SH:(c + 1) * SH]
        xT[c, :, :SH] = xs.T.astype(BF)
        dege[c, :SH] = degg[c * SH:(c + 1) * SH].astype(BF)
        bs = batch[c * SH:(c + 1) * SH]
        oh = (bs[:, None] == np.arange(G)[None, :]).astype(np.float32)
        ohp = np.zeros((SHP, G), dtype=np.float32)
        ohp[:SH] = oh
        pool_oh[c] = ohp.reshape(NB, 128, G).astype(BF)
    cnt = np.bincount(batch.astype(np.int64), minlength=G).astype(np.float32)
    inv_cnt = (1.0 / np.maximum(cnt, 1.0)).reshape(G, 1).astype(np.float32)
    key = (tuple(slots[:, 0].tolist()), tuple(slots[:, 1].tolist()))
    return key, idx_res, masks.view(BF), xT, dege, pool_oh, inv_cnt


def _build(key):
    from concourse import bass, mybir, tile, bacc
    from concourse.masks import make_identity
    F32 = mybir.dt.float32
    BF16 = mybir.dt.bfloat16
    I16 = mybir.dt.int16
    AF = mybir.ActivationFunctionType
    OP = mybir.AluOpType

    slots0, slots1 = key
    NBL = len(slots0)
    ktb = [a + b for a, b in zip(slots0, slots1)]
    T = sum(ktb)
    toff = [0]
    for k in ktb:
        toff.append(toff[-1] + k)
    KTMAX = max(ktb)
    IT = T * 8

    nc = bacc.Bacc("TRN2", target_bir_lowering=False, debug=False,
                   enable_asserts=False, num_devices=NCORES,
                   num_swdge_queues=1)

    t_xT = nc.dram_tensor("xT", [IN_C, SHP], BF16, kind="ExternalInput").ap()
    t_dege = nc.dram_tensor("dege", [SHP, HID], BF16, kind="ExternalInput").ap()
    t_idx = nc.dram_tensor("idx", [128, IT], I16, kind="ExternalInput").ap()
    t_masks = nc.dram_tensor("masks", [128, T * 256], BF16, kind="ExternalInput").ap()
    t_Win = nc.dram_tensor("Win", [IN_C, HID], BF16, kind="ExternalInput").ap()
    t_Wla = nc.dram_tensor("Wla", [L, HID, AUG], BF16, kind="ExternalInput").ap()
    t_Wr = nc.dram_tensor("Wr", [L, HID, HID], BF16, kind="ExternalInput").ap()
    t_att04 = nc.dram_tensor("att04", [L, 128, 2 * HEADS], BF16, kind="ExternalInput").ap()
    t_gam = nc.dram_tensor("gamr", [L, 128, HID], F32, kind="ExternalInput").ap()
    t_bet = nc.dram_tensor("betr", [L, 128, HID], F32, kind="ExternalInput").ap()
    t_bcv = nc.dram_tensor("bcvr", [L, 128, HID], F32, kind="ExternalInput").ap()
    t_poh = nc.dram_tensor("poh", [NBL, 128, G], BF16, kind="ExternalInput").ap()
    t_icnt = nc.dram_tensor("icnt", [G, 1], F32, kind="ExternalInput").ap()
    t_Wc = nc.dram_tensor("Wc", [HID, NCLS], BF16, kind="ExternalInput").ap()
    t_bc = nc.dram_tensor("bcr", [G, NCLS], F32, kind="ExternalInput").ap()
    t_out = nc.dram_tensor("out", [G, NCLS], F32, kind="ExternalOutput").ap()

    with tile.TileContext(nc) as tc:
        from contextlib import ExitStack
        with ExitStack() as ctx:
            cpool = ctx.enter_context(tc.tile_pool(name="const", bufs=1))
            dram = ctx.enter_context(tc.tile_pool(name="dram", bufs=1, space="DRAM"))
            sres = ctx.enter_context(tc.tile_pool(name="sres", bufs=1))
            sb = ctx.enter_context(tc.tile_pool(name="sb", bufs=3))
            gp = ctx.enter_context(tc.tile_pool(name="gp", bufs=2))
            ps = ctx.enter_context(tc.tile_pool(name="ps", bufs=2, space="PSUM"))
            pt = ctx.enter_context(tc.tile_pool(name="pt", bufs=1, space="PSUM"))
            pz = ctx.enter_context(tc.tile_pool(name="pz", bufs=2, space="PSUM"))
            pacc = ctx.enter_context(tc.tile_pool(name="pacc", bufs=2, space="PSUM"))
            ppool = ctx.enter_context(tc.tile_pool(name="ppool", bufs=1, space="PSUM"))

            identb = cpool.tile([128, 128], BF16)
            make_identity(nc, identb[:])
            epsc = cpool.tile([128, 1], F32)
            nc.vector.memset(epsc[:], 1e-5)

            idx_res = sres.tile([128, IT], I16)
            nc.sync.dma_start(idx_res[:], t_idx)
            xr_res = sres.tile([128, NBL * HID], BF16)
            h_res = sres.tile([128, NBL * HID], BF16)

            xl_sh = dram.tile([SHP, ROW], BF16)
            xl_full = dram.tile([NCORES * SHP, ROW], BF16)

            # ---- stage A: h0 = x @ Win + (b_in + deg_emb[deg]) ----
            winsb = cpool.tile([IN_C, HID], BF16)
            nc.sync.dma_start(winsb[:], t_Win)
            for nt in range(NBL):
                xt = sb.tile([IN_C, 128], BF16, tag="xt")
                nc.sync.dma_start(xt[:], t_xT[:, nt * 128:(nt + 1) * 128])
                p0 = ps.tile([128, HID], F32, tag="mm", space="PSUM")
                nc.tensor.matmul(out=p0[:], lhsT=xt[:], rhs=winsb[:],
                                 start=True, stop=True)
                dg = sb.tile([128, HID], BF16, tag="dg")
                nc.sync.dma_start(dg[:], t_dege[nt * 128:(nt + 1) * 128, :])
                nc.vector.tensor_tensor(out=h_res[:, nt * HID:(nt + 1) * HID],
                                        in0=p0[:], in1=dg[:], op=OP.add)

            for l in range(L):
                att04 = cpool.tile([128, 2 * HEADS], BF16, tag=f"att{l}")
                nc.sync.dma_start(att04[:], t_att04[l])
                gamr = cpool.tile([128, HID], F32, tag=f"gamr{l}")
                nc.sync.dma_start(gamr[:], t_gam[l])
                betr = cpool.tile([128, HID], F32, tag=f"betr{l}")
                nc.sync.dma_start(betr[:], t_bet[l])
                bcvr = cpool.tile([128, HID], F32, tag=f"bcvr{l}")
                nc.sync.dma_start(bcvr[:], t_bcv[l])
                wl_sb = cpool.tile([128, 2 * AUG], BF16, tag=f"wl{l}")
                wr_sb = cpool.tile([128, 2 * HID], BF16, tag=f"wr{l}")
                for half in range(2):
                    nc.sync.dma_start(wl_sb[:, half * AUG:(half + 1) * AUG],
                                      t_Wla[l, half * 128:(half + 1) * 128, :])
                    nc.sync.dma_start(wr_sb[:, half * HID:(half + 1) * HID],
                                      t_Wr[l, half * 128:(half + 1) * 128, :])

                # ---- B1: xl_aug / xr for own shard ----
                for nt in range(NBL):
                    hT = sb.tile([128, HID], BF16, tag="hT")
                    for half in range(2):
                        tp = pt.tile([128, 128], BF16, tag="t128", space="PSUM")
                        nc.tensor.transpose(
                            out=tp[:],
                            in_=h_res[:, nt * HID + half * 128:nt * HID + (half + 1) * 128],
                            identity=identb[:])
                        if half == 0:
                            nc.scalar.activation(out=hT[:, 0:128], in_=tp[:],
                                                 func=AF.Copy)
                        else:
                            nc.vector.tensor_copy(hT[:, 128:256], tp[:])
                    pxl = ps.tile([128, AUG], F32, tag="mm", space="PSUM")
                    pxr = ps.tile([128, HID], F32, tag="mm", space="PSUM")
                    for half in range(2):
                        nc.tensor.matmul(out=pxl[:], lhsT=hT[:, half * 128:(half + 1) * 128],
                                         rhs=wl_sb[:, half * AUG:(half + 1) * AUG],
                                         start=half == 0, stop=half == 1)
                        nc.tensor.matmul(out=pxr[:], lhsT=hT[:, half * 128:(half + 1) * 128],
                                         rhs=wr_sb[:, half * HID:(half + 1) * HID],
                                         start=half == 0, stop=half == 1)
                    xl_o = sb.tile([128, AUG], BF16, tag="xl_o")
                    nc.scalar.activation(out=xl_o[:], in_=pxl[:], func=AF.Copy)
                    nc.sync.dma_start(xl_sh[nt * 128:(nt + 1) * 128, 0:AUG], xl_o[:])
                    nc.vector.tensor_copy(xr_res[:, nt * HID:(nt + 1) * HID], pxr[:])

                # ---- B2: AllGather xl_aug ----
                nc.gpsimd.collective_compute(
                    "AllGather", OP.bypass, replica_groups=[list(range(NCORES))],
                    ins=[xl_sh[:].opt()], outs=[xl_full[:].opt()])

                # ---- B3: edge phase, software-pipelined over tiles ----
                for b in range(NBL):
                    kt = ktb[b]
                    mask_all = gp.tile([128, KTMAX * 256], BF16, tag="mask")
                    nc.sync.dma_start(
                        mask_all[:, 0:kt * 256],
                        t_masks[:, toff[b] * 256:(toff[b] + kt) * 256])
                    xl_e = gp.tile([128, KTMAX * ROW], BF16, tag="xe")
                    for h in range(2):
                        sl = (slots0[b], slots1[b])[h]
                        if sl == 0:
                            continue
                        so = slots0[b] if h else 0
                        io16 = (toff[b] + so) * 8
                        nc.gpsimd.dma_gather(
                            out_ap=xl_e[:, so * ROW:(so + sl) * ROW].rearrange(
                                "p (s e) -> p s e", e=ROW),
                            in_ap=xl_full[h * HALFR:(h + 1) * HALFR, :],
                            idxs_ap=idx_res[:, io16:io16 + sl * 8],
                            num_idxs=sl * 128, num_idxs_reg=sl * 128,
                            elem_size=ROW, queue_num=0, single_packet=False)
                    acc = pacc.tile([128, AUG], F32, tag="acc", space="PSUM")
                    xrb = xr_res[:, b * HID:(b + 1) * HID]

                    def emit_pz(t):
                        pzt = pz.tile([128, AUG], F32, tag="pzt", space="PSUM")
                        xe_t = xl_e[:, t * ROW:t * ROW + AUG]
                        m_ = mask_all[:, t * 256 + 128:(t + 1) * 256]
                        for half in range(2):
                            cs = slice(half * 128, (half + 1) * 128)
                            nc.tensor.matmul(out=pzt[:, cs], lhsT=xrb[:, cs],
                                             rhs=m_, start=True, stop=False)
                            nc.tensor.matmul(out=pzt[:, cs], lhsT=xe_t[:, cs],
                                             rhs=identb[:], start=False, stop=True)
                        azt = sb.tile([128, HID], BF16, tag="azt")
                        nc.scalar.activation(out=azt[:], in_=pzt[:, 0:HID], func=AF.Abs)
                        return pzt, azt

                    def emit_rest(t, pzt, azt):
                        xe_t = xl_e[:, t * ROW:t * ROW + AUG]
                        m2 = mask_all[:, t * 256:t * 256 + 128]
                        lg = pzt[:, HID:AUG]
                        nc.tensor.matmul(out=lg, lhsT=azt[:, 0:128],
                                         rhs=att04[:, 0:HEADS], start=True, stop=False)
                        nc.tensor.matmul(out=lg, lhsT=azt[:, 128:256],
                                         rhs=att04[:, HEADS:2 * HEADS],
                                         start=False, stop=False)
                        nc.tensor.matmul(out=lg, lhsT=identb[:], rhs=xe_t[:, HID:AUG],
                                         start=False, stop=True)
                        wm = sb.tile([128, AUG], BF16, tag="wm")
                        nc.scalar.activation(out=wm[:, HID:AUG], in_=lg, func=AF.Exp)
                        nc.vector.tensor_tensor(
                            out=wm[:, 0:HID].rearrange("p (h d) -> p h d", h=HEADS),
                            in0=xe_t[:, 0:HID].rearrange("p (h d) -> p h d", h=HEADS),
                            in1=wm[:, HID:AUG].unsqueeze(2).broadcast_to([128, HEADS, HD]),
                            op=OP.mult)
                        nc.tensor.matmul(out=acc[:], lhsT=m2, rhs=wm[:],
                                         start=t == 0, stop=t == kt - 1)

                    prev = None
                    for t in range(kt + 1):
                        if t < kt:
                            cur = emit_pz(t)
                        if prev is not None:
                            emit_rest(t - 1, *prev)
                        prev = cur if t < kt else None

                    # ---- finalize block: softmax div, bias, relu, residual, LN
                    rcp = sb.tile([128, HEADS], F32, tag="rcp")
                    nc.vector.reciprocal(rcp[:], acc[:, HID:AUG])
                    v0 = sb.tile([128, HID], F32, tag="v0")
                    nc.vector.tensor_tensor(
                        out=v0[:].rearrange("p (h d) -> p h d", h=HEADS),
                        in0=acc[:, 0:HID].rearrange("p (h d) -> p h d", h=HEADS),
                        in1=rcp[:].unsqueeze(2).broadcast_to([128, HEADS, HD]),
                        op=OP.mult)
                    nc.vector.tensor_tensor(out=v0[:], in0=v0[:], in1=bcvr[:], op=OP.add)
                    v = sb.tile([128, HID], F32, tag="v")
                    msum = sb.tile([128, 1], F32, tag="msum")
                    nc.vector.scalar_tensor_tensor(
                        out=v[:], in0=v0[:], scalar=0.0,
                        in1=h_res[:, b * HID:(b + 1) * HID],
                        op0=OP.max, op1=OP.add, accum_out=msum[:])
                    scr = sb.tile([128, HID], F32, tag="scr")
                    ssq = sb.tile([128, 1], F32, tag="ssq")
                    nc.scalar.activation(out=scr[:], in_=v[:], func=AF.Square,
                                         accum_out=ssq[:])
                    mu = sb.tile([128, 1], F32, tag="mu")
                    nc.vector.tensor_scalar(out=mu[:], in0=msum[:], scalar1=1.0 / HID,
                                            scalar2=None, op0=OP.mult)
                    mu2 = sb.tile([128, 1], F32, tag="mu2")
                    nc.vector.tensor_tensor(out=mu2[:], in0=mu[:], in1=mu[:], op=OP.mult)
                    var = sb.tile([128, 1], F32, tag="var")
                    nc.vector.tensor_scalar(out=var[:], in0=ssq[:], scalar1=1.0 / HID,
                                            scalar2=mu2[:, :1], op0=OP.mult,
                                            op1=OP.subtract)
                    std = sb.tile([128, 1], F32, tag="std")
                    nc.scalar.activation(out=std[:], in_=var[:], func=AF.Sqrt,
                                         bias=epsc[:, :1])
                    rstd = sb.tile([128, 1], F32, tag="rstd")
                    nc.vector.reciprocal(rstd[:], std[:])
                    t1 = sb.tile([128, HID], F32, tag="t1")
                    nc.vector.tensor_scalar(out=t1[:], in0=v[:], scalar1=mu[:, :1],
                                            scalar2=rstd[:, :1], op0=OP.subtract,
                                            op1=OP.mult)
                    t2 = sb.tile([128, HID], F32, tag="t2")
                    nc.vector.tensor_tensor(out=t2[:], in0=t1[:], in1=gamr[:], op=OP.mult)
                    nc.vector.tensor_tensor(out=h_res[:, b * HID:(b + 1) * HID],
                                            in0=t2[:], in1=betr[:], op=OP.add)
                    if l == L - 1:
                        poh = sb.tile([128, G], BF16, tag="poh")
                        nc.sync.dma_start(poh[:], t_poh[b])
                        if b == 0:
                            ppool_t = ppool.tile([G, HID], F32, tag="ppool",
                                                 space="PSUM")
                            _build.ppool_t = ppool_t
                        nc.tensor.matmul(out=_build.ppool_t[:], lhsT=poh[:],
                                         rhs=h_res[:, b * HID:(b + 1) * HID],
                                         start=b == 0, stop=b == NBL - 1)

            # ---- stage C: local pool @ Wc, AllReduce tiny logits ----
            pool_sb = sb.tile([G, HID], BF16, tag="pool_sb")
            nc.scalar.activation(out=pool_sb[:], in_=_build.ppool_t[:], func=AF.Copy)
            wc_sb = cpool.tile([128, 2 * NCLS], BF16, tag="wc")
            for half in range(2):
                nc.sync.dma_start(wc_sb[:, half * NCLS:(half + 1) * NCLS],
                                  t_Wc[half * 128:(half + 1) * 128, :])
            pcls = pacc.tile([G, NCLS], F32, tag="acc", space="PSUM")
            for half in range(2):
                tp = pt.tile([128, G], BF16, tag="t128", space="PSUM")
                nc.tensor.transpose(out=tp[:], in_=pool_sb[:, half * 128:(half + 1) * 128],
                                    identity=identb[:G, :G])
                ptc = sb.tile([128, G], BF16, tag="ptc")
                nc.scalar.activation(out=ptc[:], in_=tp[:], func=AF.Copy)
                nc.tensor.matmul(out=pcls[:], lhsT=ptc[:],
                                 rhs=wc_sb[:, half * NCLS:(half + 1) * NCLS],
                                 start=half == 0, stop=half == 1)
            z_sb = sb.tile([G, NCLS], F32, tag="z_sb")
            nc.scalar.activation(out=z_sb[:], in_=pcls[:], func=AF.Copy)
            zc = dram.tile([G, NCLS], F32)
            zr = dram.tile([G, NCLS], F32)
            nc.sync.dma_start(zc[:], z_sb[:])
            nc.gpsimd.collective_compute(
                "AllReduce", OP.add, replica_groups=[list(range(NCORES))],
                ins=[zc[:].opt()], outs=[zr[:].opt()])
            zr_sb = sb.tile([G, NCLS], F32, tag="zr_sb")
            nc.sync.dma_start(zr_sb[:], zr[:])
            icnt = sb.tile([G, 1], F32, tag="icnt")
            nc.sync.dma_start(icnt[:], t_icnt)
            bc_sb = sb.tile([G, NCLS], F32, tag="bc")
            nc.sync.dma_start(bc_sb[:], t_bc)
            res = sb.tile([G, NCLS], F32, tag="resout")
            nc.vector.tensor_scalar(out=res[:], in0=zr_sb[:], scalar1=icnt[:, :1],
                                    scalar2=None, op0=OP.mult)
            nc.vector.tensor_tensor(out=res[:], in0=res[:], in1=bc_sb[:], op=OP.add)
            nc.sync.dma_start(t_out, res[:])

    nc.compile()
    return nc


def _prep_weights(Win, Wl, Wr, att, b_conv, gamma, beta, Wc, b_c):
    Win_b = Win.astype(BF)
    Wla = np.zeros((L, HID, AUG), dtype=BF)
    att04 = np.zeros((L, 128, 2 * HEADS), dtype=BF)
    for l in range(L):
        Wla[l, :, :HID] = Wl[l].astype(BF)
        for h in range(HEADS):
            Wla[l, :, HID + h] = (0.6 * Wl[l][:, h * HD:(h + 1) * HD] @ att[l, h]).astype(BF)
        bd = np.zeros((HID, HEADS), dtype=np.float32)
        for h in range(HEADS):
            bd[h * HD:(h + 1) * HD, h] = 0.4 * att[l, h]
        att04[l, :, :HEADS] = bd[:128].astype(BF)
        att04[l, :, HEADS:] = bd[128:].astype(BF)
    gam_rep = np.repeat(gamma.reshape(L, 1, HID), 128, axis=1).astype(np.float32)
    bet_rep = np.repeat(beta.reshape(L, 1, HID), 128, axis=1).astype(np.float32)
    bcv_rep = np.repeat(b_conv.reshape(L, 1, HID), 128, axis=1).astype(np.float32)
    bc_rep = np.repeat(b_c.reshape(1, NCLS), G, axis=0).astype(np.float32)
    return (Win_b, Wla, Wr.astype(BF), att04, gam_rep, bet_rep, bcv_rep,
            Wc.astype(BF), bc_rep)


def make_in_maps(inputs):
    x = np.asarray(inputs["x"], dtype=np.float32)
    edge_index = np.asarray(inputs["edge_index"])
    batch = np.asarray(inputs["batch"])
    deg = np.asarray(inputs["deg"])
    Win = np.asarray(inputs["Win"], dtype=np.float32)
    b_in = np.asarray(inputs["b_in"], dtype=np.float32)
    deg_emb = np.asarray(inputs["deg_emb"], dtype=np.float32)
    Wl = np.asarray(inputs["Wl"], dtype=np.float32)
    Wr = np.asarray(inputs["Wr"], dtype=np.float32)
    att = np.asarray(inputs["att"], dtype=np.float32)
    b_conv = np.asarray(inputs["b_conv"], dtype=np.float32)
    gamma = np.asarray(inputs["gamma"], dtype=np.float32)
    beta = np.asarray(inputs["beta"], dtype=np.float32)
    Wc = np.asarray(inputs["Wc"], dtype=np.float32)
    b_c = np.asarray(inputs["b_c"], dtype=np.float32)

    key, idx_res, masks, xT, dege, pool_oh, inv_cnt = _host_prep(
        edge_index, batch, deg, deg_emb, x, b_in)
    Win_b, Wla, Wr_b, att04, gam_rep, bet_rep, bcv_rep, Wc_b, bc_rep = _prep_weights(
        Win, Wl, Wr, att, b_conv, gamma, beta, Wc, b_c)

    in_maps = []
    for c in range(NCORES):
        in_maps.append({
            "xT": xT[c], "dege": dege[c], "idx": idx_res[c], "masks": masks[c],
            "Win": Win_b, "Wla": Wla, "Wr": Wr_b, "att04": att04,
            "gamr": gam_rep, "betr": bet_rep, "bcvr": bcv_rep,
            "poh": pool_oh[c], "icnt": inv_cnt, "Wc": Wc_b, "bcr": bc_rep,
        })
    return key, in_maps


def kernel(**inputs):
    from concourse import bass_utils
    key, in_maps = make_in_maps(inputs)
    if key not in _CACHE:
        _CACHE[key] = _build(key)
    nc = _CACHE[key]
    res = bass_utils.run_bass_kernel_spmd(nc, in_maps, core_ids=list(range(NCORES)))
    kernel.last_results = res
    return res.results[0]["out"].astype(np.float32)


# revision 13
# speedup vs baseline: 3.1183x; 1.0645x over previous
"""GATv2 GNN (Graphormer-style) on 8 trn2 NeuronCores.

Strategy: edges sorted by destination, sharded by destination range
(6250 dsts/core -> 49 blocks of 128 dsts). Per layer: each core computes
its shard of xl_aug=[h@Wl | 0.6*att.xl] (384-col padded rows) and xr,
AllGathers xl_aug (bf16), then per dst block gathers all its edges'
xl_aug[src] rows with TWO dma_gather instructions (int16 indices limit
one gather to 25088 rows, so the node table is split in halves) plus a
precomputed one-hot mask load. Per 128-edge tile attention is pure
matmuls: leaky_relu(p) = 0.6*p + 0.4*|p| so
  logit = 0.6*att.xl[src] + att04 . |pz|   (the xr-side linear term is
constant per destination and cancels in the softmax). pz^T is formed by
mask matmuls + transpose-accumulate; messages scatter-add into PSUM via
the one-hot mask matmul. All matmul data bf16; accumulation fp32.
"""
import sys
import numpy as np
import ml_dtypes

sys.path.insert(0, '/opt/trn_rl_repo')

BF = ml_dtypes.bfloat16

N, E, IN_C, HID, HEADS, L, G, NCLS, MAXDEG = 50000, 800000, 128, 256, 4, 2, 64, 10, 10
HD = HID // HEADS
NCORES = 8
SH = N // NCORES            # 6250 real nodes per shard
NB = 49                     # dst blocks per core (49*128 = 6272 padded)
SHP = NB * 128
AUG = HID + HEADS           # 260 useful cols
ROW = 384                   # padded gather row (768B, 256B-multiple)
HALFR = NCORES * SHP // 2   # 25088 rows per gather table half (int16 range)
AGC = [13, 12, 12, 12]      # AllGather chunks, in dst blocks
AGO = [0, 13, 25, 37]       # chunk block offsets


def _remap_rows(src):
    """Node id -> row in the chunked-AllGather xl_full layout."""
    c = src // SH
    local = src % SH
    blk = local // 128
    # chunk index of this block
    ch = np.searchsorted(np.cumsum(AGC), blk, side='right')
    base = np.concatenate([[0], np.cumsum(np.array(AGC) * 128 * NCORES)])[ch]
    off = (blk - np.array(AGO)[ch]) * 128 + (local % 128)
    return (base + c * np.array(AGC)[ch] * 128 + off).astype(np.int64)

_CACHE = {}


def _host_prep(edge_index, batch, deg, deg_emb, x, b_in):
    src = np.concatenate([edge_index[0], np.arange(N, dtype=np.int64)]).astype(np.int64)
    dst = np.concatenate([edge_index[1], np.arange(N, dtype=np.int64)]).astype(np.int64)
    order = np.argsort(dst, kind='stable')
    src, dst = src[order], dst[order]
    src_remap = _remap_rows(src)
    core_of = (dst // SH).astype(np.int64)
    loc = (dst % SH).astype(np.int64)
    lblk = loc // 128

    # collect per (core, block, half) edge lists
    ed = [[[None, None] for _ in range(NB)] for _ in range(NCORES)]
    cnts = np.zeros((NCORES, NB, 2), dtype=np.int64)
    for c in range(NCORES):
        m = core_of == c
        s_c, l_c, b_c = src_remap[m], loc[m], lblk[m]
        o2 = np.argsort(b_c, kind='stable')
        s_c, l_c, b_c = s_c[o2], l_c[o2], b_c[o2]
        starts = np.searchsorted(b_c, np.arange(NB))
        ends = np.searchsorted(b_c, np.arange(NB) + 1)
        for b in range(NB):
            sl = slice(starts[b], ends[b])
            ss = s_c[sl]
            dl = l_c[sl] - b * 128
            if b == NB - 1:
                # fake self-edges for the 22 padding dsts: keeps softmax
                # denominator > 0 so LayerNorm of padding rows stays finite
                pads = np.arange(SH % 128, 128, dtype=np.int64)
                ss = np.concatenate([ss, np.zeros(len(pads), np.int64)])
                dl = np.concatenate([dl, pads])
            hm = ss >= HALFR
            for h in range(2):
                sel = hm if h else ~hm
                ed[c][b][h] = (ss[sel] - h * HALFR, dl[sel])
                cnts[c, b, h] = sel.sum()

    slots = np.maximum(0, np.ceil(cnts.max(axis=0) / 128)).astype(np.int64)  # [NB,2]
    ktb = (slots[:, 0] + slots[:, 1]).astype(np.int64)
    T = int(ktb.sum())
    toff = np.zeros(NB + 1, dtype=np.int64)
    toff[1:] = np.cumsum(ktb)
    IT = T * 8   # int16 idx cols: per slot-tile 128 idxs -> 8 cols of 16 rows

    idx16 = np.zeros((NCORES, 16, IT), dtype=np.int16)
    masks = np.zeros((NCORES, 128, T * 256), dtype=np.uint16)
    ONE = np.float32(1.0).astype(BF).view(np.uint16)
    for c in range(NCORES):
        for b in range(NB):
            for h in range(2):
                sl = int(slots[b, h])
                if sl == 0:
                    continue
                t0 = toff[b] + (slots[b, 0] if h else 0)
                ss, dl = ed[c][b][h]
                n = len(ss)
                L128 = sl * 128
                idpad = np.zeros(L128, dtype=np.int16)
                idpad[:n] = ss.astype(np.int16)
                # wrapped layout: idx j -> [j%16, j//16]
                idx16[c, :, t0 * 8:(t0 + sl) * 8] = idpad.reshape(-1, 16).T
                j = np.arange(n)
                tt = t0 + j // 128
                e = j % 128
                masks[c, e, tt * 256 + dl] = ONE            # m2[e, d]
                masks[c, dl, tt * 256 + 128 + e] = ONE      # m_[d, e]

    idx_res = np.tile(idx16, (1, 8, 1))  # replicate into 128 partitions

    degg = (deg_emb[np.clip(deg, 0, MAXDEG)] + b_in[None, :]).astype(np.float32)
    xT = np.zeros((NCORES, IN_C, SHP), dtype=BF)
    dege = np.zeros((NCORES, SHP, HID), dtype=BF)
    pool_oh = np.zeros((NCORES, NB, 128, G), dtype=BF)
    for c in range(NCORES):
        xs = x[c * SH:(c + 1) * SH]
        xT[c, :, :SH] = xs.T.astype(BF)
        dege[c, :SH] = degg[c * SH:(c + 1) * SH].astype(BF)
        bs = batch[c * SH:(c + 1) * SH]
        oh = (bs[:, None] == np.arange(G)[None, :]).astype(np.float32)
        ohp = np.zeros((SHP, G), dtype=np.float32)
        ohp[:SH] = oh
        pool_oh[c] = ohp.reshape(NB, 128, G).astype(BF)
    cnt = np.bincount(batch.astype(np.int64), minlength=G).astype(np.float32)
    inv_cnt = (1.0 / np.maximum(cnt, 1.0)).reshape(G, 1).astype(np.float32)
    key = (tuple(slots[:, 0].tolist()), tuple(slots[:, 1].tolist()))
    return key, idx_res, masks.view(BF), xT, dege, pool_oh, inv_cnt


def _build(key):
    from concourse import bass, mybir, tile, bacc
    from concourse.masks import make_identity
    F32 = mybir.dt.float32
    BF16 = mybir.dt.bfloat16
    I16 = mybir.dt.int16
    AF = mybir.ActivationFunctionType
    OP = mybir.AluOpType

    slots0, slots1 = key
    NBL = len(slots0)
    ktb = [a + b for a, b in zip(slots0, slots1)]
    T = sum(ktb)
    toff = [0]
    for k in ktb:
        toff.append(toff[-1] + k)
    KTMAX = max(ktb)
    IT = T * 8

    nc = bacc.Bacc("TRN2", target_bir_lowering=False, debug=False,
                   enable_asserts=False, num_devices=NCORES,
                   num_swdge_queues=1)

    t_xT = nc.dram_tensor("xT", [IN_C, SHP], BF16, kind="ExternalInput").ap()
    t_dege = nc.dram_tensor("dege", [SHP, HID], BF16, kind="ExternalInput").ap()
    t_idx = nc.dram_tensor("idx", [128, IT], I16, kind="ExternalInput").ap()
    t_masks = nc.dram_tensor("masks", [128, T * 256], BF16, kind="ExternalInput").ap()
    t_Win = nc.dram_tensor("Win", [IN_C, HID], BF16, kind="ExternalInput").ap()
    t_Wla = nc.dram_tensor("Wla", [L, HID, AUG], BF16, kind="ExternalInput").ap()
    t_Wr = nc.dram_tensor("Wr", [L, HID, HID], BF16, kind="ExternalInput").ap()
    t_att04 = nc.dram_tensor("att04", [L, 128, 2 * HEADS], BF16, kind="ExternalInput").ap()
    t_gam = nc.dram_tensor("gamr", [L, 128, HID], F32, kind="ExternalInput").ap()
    t_bet = nc.dram_tensor("betr", [L, 128, HID], F32, kind="ExternalInput").ap()
    t_bcv = nc.dram_tensor("bcvr", [L, 128, HID], F32, kind="ExternalInput").ap()
    t_poh = nc.dram_tensor("poh", [NBL, 128, G], BF16, kind="ExternalInput").ap()
    t_icnt = nc.dram_tensor("icnt", [G, 1], F32, kind="ExternalInput").ap()
    t_Wc = nc.dram_tensor("Wc", [HID, NCLS], BF16, kind="ExternalInput").ap()
    t_bc = nc.dram_tensor("bcr", [G, NCLS], F32, kind="ExternalInput").ap()
    t_out = nc.dram_tensor("out", [G, NCLS], F32, kind="ExternalOutput").ap()

    with tile.TileContext(nc) as tc:
        from contextlib import ExitStack
        with ExitStack() as ctx:
            cpool = ctx.enter_context(tc.tile_pool(name="const", bufs=1))
            dram = ctx.enter_context(tc.tile_pool(name="dram", bufs=1, space="DRAM"))
            sres = ctx.enter_context(tc.tile_pool(name="sres", bufs=1))
            sb = ctx.enter_context(tc.tile_pool(name="sb", bufs=3))
            gp = ctx.enter_context(tc.tile_pool(name="gp", bufs=3))
            ps = ctx.enter_context(tc.tile_pool(name="ps", bufs=2, space="PSUM"))
            pt = ctx.enter_context(tc.tile_pool(name="pt", bufs=1, space="PSUM"))
            pz = ctx.enter_context(tc.tile_pool(name="pz", bufs=2, space="PSUM"))
            pacc = ctx.enter_context(tc.tile_pool(name="pacc", bufs=2, space="PSUM"))
            ppool = ctx.enter_context(tc.tile_pool(name="ppool", bufs=1, space="PSUM"))

            identb = cpool.tile([128, 128], BF16)
            make_identity(nc, identb[:])
            epsc = cpool.tile([128, 1], F32)
            nc.vector.memset(epsc[:], 1e-5)

            idx_res = sres.tile([128, IT], I16)
            nc.sync.dma_start(idx_res[:], t_idx)
            xr_res = sres.tile([128, NBL * HID], BF16)
            h_res = sres.tile([128, NBL * HID], BF16)

            xl_sh = dram.tile([SHP, ROW], BF16)
            xl_full = dram.tile([NCORES * SHP, ROW], BF16)

            # ---- stage A: h0 = x @ Win + (b_in + deg_emb[deg]) ----
            winsb = cpool.tile([IN_C, HID], BF16)
            nc.sync.dma_start(winsb[:], t_Win)
            for nt in range(NBL):
                xt = sb.tile([IN_C, 128], BF16, tag="xt")
                nc.sync.dma_start(xt[:], t_xT[:, nt * 128:(nt + 1) * 128])
                p0 = ps.tile([128, HID], F32, tag="mm", space="PSUM")
                nc.tensor.matmul(out=p0[:], lhsT=xt[:], rhs=winsb[:],
                                 start=True, stop=True)
                dg = sb.tile([128, HID], BF16, tag="dg")
                nc.sync.dma_start(dg[:], t_dege[nt * 128:(nt + 1) * 128, :])
                nc.vector.tensor_tensor(out=h_res[:, nt * HID:(nt + 1) * HID],
                                        in0=p0[:], in1=dg[:], op=OP.add)

            for l in range(L):
                att04 = cpool.tile([128, 2 * HEADS], BF16, tag=f"att{l}")
                nc.sync.dma_start(att04[:], t_att04[l])
                gamr = cpool.tile([128, HID], F32, tag=f"gamr{l}")
                nc.sync.dma_start(gamr[:], t_gam[l])
                betr = cpool.tile([128, HID], F32, tag=f"betr{l}")
                nc.sync.dma_start(betr[:], t_bet[l])
                bcvr = cpool.tile([128, HID], F32, tag=f"bcvr{l}")
                nc.sync.dma_start(bcvr[:], t_bcv[l])
                wl_sb = cpool.tile([128, 2 * AUG], BF16, tag=f"wl{l}")
                wr_sb = cpool.tile([128, 2 * HID], BF16, tag=f"wr{l}")
                for half in range(2):
                    nc.sync.dma_start(wl_sb[:, half * AUG:(half + 1) * AUG],
                                      t_Wla[l, half * 128:(half + 1) * 128, :])
                    nc.sync.dma_start(wr_sb[:, half * HID:(half + 1) * HID],
                                      t_Wr[l, half * 128:(half + 1) * 128, :])

                # ---- B1: xl_aug / xr for own shard ----
                for nt in range(NBL):
                    hT = sb.tile([128, HID], BF16, tag="hT")
                    for half in range(2):
                        tp = pt.tile([128, 128], BF16, tag="t128", space="PSUM")
                        nc.tensor.transpose(
                            out=tp[:],
                            in_=h_res[:, nt * HID + half * 128:nt * HID + (half + 1) * 128],
                            identity=identb[:])
                        if half == 0:
                            nc.scalar.activation(out=hT[:, 0:128], in_=tp[:],
                                                 func=AF.Copy)
                        else:
                            nc.vector.tensor_copy(hT[:, 128:256], tp[:])
                    pxl = ps.tile([128, AUG], F32, tag="mm", space="PSUM")
                    pxr = ps.tile([128, HID], F32, tag="mm", space="PSUM")
                    for half in range(2):
                        nc.tensor.matmul(out=pxl[:], lhsT=hT[:, half * 128:(half + 1) * 128],
                                         rhs=wl_sb[:, half * AUG:(half + 1) * AUG],
                                         start=half == 0, stop=half == 1)
                        nc.tensor.matmul(out=pxr[:], lhsT=hT[:, half * 128:(half + 1) * 128],
                                         rhs=wr_sb[:, half * HID:(half + 1) * HID],
                                         start=half == 0, stop=half == 1)
                    xl_o = sb.tile([128, AUG], BF16, tag="xl_o")
                    nc.scalar.activation(out=xl_o[:], in_=pxl[:], func=AF.Copy)
                    nc.sync.dma_start(xl_sh[nt * 128:(nt + 1) * 128, 0:AUG], xl_o[:])
                    nc.vector.tensor_copy(xr_res[:, nt * HID:(nt + 1) * HID], pxr[:])
                    # ---- B2: chunked AllGather, overlapped with B1 tail ----
                    for k in range(len(AGC)):
                        if nt == AGO[k] + AGC[k] - 1:
                            r0 = AGO[k] * 128
                            rn = AGC[k] * 128
                            fb = sum(AGC[j] * 128 * NCORES for j in range(k))
                            nc.gpsimd.collective_compute(
                                "AllGather", OP.bypass,
                                replica_groups=[list(range(NCORES))],
                                ins=[xl_sh[r0:r0 + rn, :].opt()],
                                outs=[xl_full[fb:fb + rn * NCORES, :].opt()])

                # ---- B3: edge phase, software-pipelined over tiles ----
                for b in range(NBL):
                    kt = ktb[b]
                    mask_all = gp.tile([128, KTMAX * 256], BF16, tag="mask")
                    nc.sync.dma_start(
                        mask_all[:, 0:kt * 256],
                        t_masks[:, toff[b] * 256:(toff[b] + kt) * 256])
                    xl_e = gp.tile([128, KTMAX * ROW], BF16, tag="xe")
                    for h in range(2):
                        sl = (slots0[b], slots1[b])[h]
                        if sl == 0:
                            continue
                        so = slots0[b] if h else 0
                        io16 = (toff[b] + so) * 8
                        nc.gpsimd.dma_gather(
                            out_ap=xl_e[:, so * ROW:(so + sl) * ROW].rearrange(
                                "p (s e) -> p s e", e=ROW),
                            in_ap=xl_full[h * HALFR:(h + 1) * HALFR, :],
                            idxs_ap=idx_res[:, io16:io16 + sl * 8],
                            num_idxs=sl * 128, num_idxs_reg=sl * 128,
                            elem_size=ROW, queue_num=0, single_packet=False)
                    acc = pacc.tile([128, AUG], F32, tag="acc", space="PSUM")
                    xrb = xr_res[:, b * HID:(b + 1) * HID]

                    def emit_pz(t):
                        pzt = pz.tile([128, AUG], F32, tag="pzt", space="PSUM")
                        xe_t = xl_e[:, t * ROW:t * ROW + AUG]
                        m_ = mask_all[:, t * 256 + 128:(t + 1) * 256]
                        for half in range(2):
                            cs = slice(half * 128, (half + 1) * 128)
                            nc.tensor.matmul(out=pzt[:, cs], lhsT=xrb[:, cs],
                                             rhs=m_, start=True, stop=False)
                            nc.tensor.matmul(out=pzt[:, cs], lhsT=xe_t[:, cs],
                                             rhs=identb[:], start=False, stop=True)
                        azt = sb.tile([128, HID], BF16, tag="azt")
                        nc.scalar.activation(out=azt[:], in_=pzt[:, 0:HID], func=AF.Abs)
                        return pzt, azt

                    def emit_rest(t, pzt, azt):
                        xe_t = xl_e[:, t * ROW:t * ROW + AUG]
                        m2 = mask_all[:, t * 256:t * 256 + 128]
                        lg = pzt[:, HID:AUG]
                        nc.tensor.matmul(out=lg, lhsT=azt[:, 0:128],
                                         rhs=att04[:, 0:HEADS], start=True, stop=False)
                        nc.tensor.matmul(out=lg, lhsT=azt[:, 128:256],
                                         rhs=att04[:, HEADS:2 * HEADS],
                                         start=False, stop=False)
                        nc.tensor.matmul(out=lg, lhsT=identb[:], rhs=xe_t[:, HID:AUG],
                                         start=False, stop=True)
                        wm = sb.tile([128, AUG], BF16, tag="wm")
                        nc.scalar.activation(out=wm[:, HID:AUG], in_=lg, func=AF.Exp)
                        nc.vector.tensor_tensor(
                            out=wm[:, 0:HID].rearrange("p (h d) -> p h d", h=HEADS),
                            in0=xe_t[:, 0:HID].rearrange("p (h d) -> p h d", h=HEADS),
                            in1=wm[:, HID:AUG].unsqueeze(2).broadcast_to([128, HEADS, HD]),
                            op=OP.mult)
                        nc.tensor.matmul(out=acc[:], lhsT=m2, rhs=wm[:],
                                         start=t == 0, stop=t == kt - 1)

                    prev = None
                    for t in range(kt + 1):
                        if t < kt:
                            cur = emit_pz(t)
                        if prev is not None:
                            emit_rest(t - 1, *prev)
                        prev = cur if t < kt else None

                    # ---- finalize block: softmax div, bias, relu, residual, LN
                    rcp = sb.tile([128, HEADS], F32, tag="rcp")
                    nc.vector.reciprocal(rcp[:], acc[:, HID:AUG])
                    v0 = sb.tile([128, HID], F32, tag="v0")
                    nc.vector.tensor_tensor(
                        out=v0[:].rearrange("p (h d) -> p h d", h=HEADS),
                        in0=acc[:, 0:HID].rearrange("p (h d) -> p h d", h=HEADS),
                        in1=rcp[:].unsqueeze(2).broadcast_to([128, HEADS, HD]),
                        op=OP.mult)
                    nc.vector.tensor_tensor(out=v0[:], in0=v0[:], in1=bcvr[:], op=OP.add)
                    v = sb.tile([128, HID], F32, tag="v")
                    msum = sb.tile([128, 1], F32, tag="msum")
                    nc.vector.scalar_tensor_tensor(
                        out=v[:], in0=v0[:], scalar=0.0,
                        in1=h_res[:, b * HID:(b + 1) * HID],
                        op0=OP.max, op1=OP.add, accum_out=msum[:])
                    scr = sb.tile([128, HID], F32, tag="scr")
                    ssq = sb.tile([128, 1], F32, tag="ssq")
                    nc.scalar.activation(out=scr[:], in_=v[:], func=AF.Square,
                                         accum_out=ssq[:])
                    mu = sb.tile([128, 1], F32, tag="mu")
                    nc.vector.tensor_scalar(out=mu[:], in0=msum[:], scalar1=1.0 / HID,
                                            scalar2=None, op0=OP.mult)
                    mu2 = sb.tile([128, 1], F32, tag="mu2")
                    nc.vector.tensor_tensor(out=mu2[:], in0=mu[:], in1=mu[:], op=OP.mult)
                    var = sb.tile([128, 1], F32, tag="var")
                    nc.vector.tensor_scalar(out=var[:], in0=ssq[:], scalar1=1.0 / HID,
                                            scalar2=mu2[:, :1], op0=OP.mult,
                                            op1=OP.subtract)
                    std = sb.tile([128, 1], F32, tag="std")
                    nc.scalar.activation(out=std[:], in_=var[:], func=AF.Sqrt,
                                         bias=epsc[:, :1])
                    rstd = sb.tile([128, 1], F32, tag="rstd")
                    nc.vector.reciprocal(rstd[:], std[:])
                    t1 = sb.tile([128, HID], F32, tag="t1")
                    nc.vector.tensor_scalar(out=t1[:], in0=v[:], scalar1=mu[:, :1],
                                            scalar2=rstd[:, :1], op0=OP.subtract,
                                            op1=OP.mult)
                    t2 = sb.tile([128, HID], F32, tag="t2")
                    nc.vector.tensor_tensor(out=t2[:], in0=t1[:], in1=gamr[:], op=OP.mult)
                    nc.vector.tensor_tensor(out=h_res[:, b * HID:(b + 1) * HID],
                                            in0=t2[:], in1=betr[:], op=OP.add)
                    if l == L - 1:
                        poh = sb.tile([128, G], BF16, tag="poh")
                        nc.sync.dma_start(poh[:], t_poh[b])
                        if b == 0:
                            ppool_t = ppool.tile([G, HID], F32, tag="ppool",
                                                 space="PSUM")
                            _build.ppool_t = ppool_t
                        nc.tensor.matmul(out=_build.ppool_t[:], lhsT=poh[:],
                                         rhs=h_res[:, b * HID:(b + 1) * HID],
                                         start=b == 0, stop=b == NBL - 1)

            # ---- stage C: local pool @ Wc, AllReduce tiny logits ----
            pool_sb = sb.tile([G, HID], BF16, tag="pool_sb")
            nc.scalar.activation(out=pool_sb[:], in_=_build.ppool_t[:], func=AF.Copy)
            wc_sb = cpool.tile([128, 2 * NCLS], BF16, tag="wc")
            for half in range(2):
                nc.sync.dma_start(wc_sb[:, half * NCLS:(half + 1) * NCLS],
                                  t_Wc[half * 128:(half + 1) * 128, :])
            pcls = pacc.tile([G, NCLS], F32, tag="acc", space="PSUM")
            for half in range(2):
                tp = pt.tile([128, G], BF16, tag="t128", space="PSUM")
                nc.tensor.transpose(out=tp[:], in_=pool_sb[:, half * 128:(half + 1) * 128],
                                    identity=identb[:G, :G])
                ptc = sb.tile([128, G], BF16, tag="ptc")
                nc.scalar.activation(out=ptc[:], in_=tp[:], func=AF.Copy)
                nc.tensor.matmul(out=pcls[:], lhsT=ptc[:],
                                 rhs=wc_sb[:, half * NCLS:(half + 1) * NCLS],
                                 start=half == 0, stop=half == 1)
            z_sb = sb.tile([G, NCLS], F32, tag="z_sb")
            nc.scalar.activation(out=z_sb[:], in_=pcls[:], func=AF.Copy)
            zc = dram.tile([G, NCLS], F32)
            zr = dram.tile([G, NCLS], F32)
            nc.sync.dma_start(zc[:], z_sb[:])
            nc.gpsimd.collective_compute(
                "AllReduce", OP.add, replica_groups=[list(range(NCORES))],
                ins=[zc[:].opt()], outs=[zr[:].opt()])
            zr_sb = sb.tile([G, NCLS], F32, tag="zr_sb")
            nc.sync.dma_start(zr_sb[:], zr[:])
            icnt = sb.tile([G, 1], F32, tag="icnt")
            nc.sync.dma_start(icnt[:], t_icnt)
            bc_sb = sb.tile([G, NCLS], F32, tag="bc")
            nc.sync.dma_start(bc_sb[:], t_bc)
            res = sb.tile([G, NCLS], F32, tag="resout")
            nc.vector.tensor_scalar(out=res[:], in0=zr_sb[:], scalar1=icnt[:, :1],
                                    scalar2=None, op0=OP.mult)
            nc.vector.tensor_tensor(out=res[:], in0=res[:], in1=bc_sb[:], op=OP.add)
            nc.sync.dma_start(t_out, res[:])

    nc.compile()
    return nc


def _prep_weights(Win, Wl, Wr, att, b_conv, gamma, beta, Wc, b_c):
    Win_b = Win.astype(BF)
    Wla = np.zeros((L, HID, AUG), dtype=BF)
    att04 = np.zeros((L, 128, 2 * HEADS), dtype=BF)
    for l in range(L):
        Wla[l, :, :HID] = Wl[l].astype(BF)
        for h in range(HEADS):
            Wla[l, :, HID + h] = (0.6 * Wl[l][:, h * HD:(h + 1) * HD] @ att[l, h]).astype(BF)
        bd = np.zeros((HID, HEADS), dtype=np.float32)
        for h in range(HEADS):
            bd[h * HD:(h + 1) * HD, h] = 0.4 * att[l, h]
        att04[l, :, :HEADS] = bd[:128].astype(BF)
        att04[l, :, HEADS:] = bd[128:].astype(BF)
    gam_rep = np.repeat(gamma.reshape(L, 1, HID), 128, axis=1).astype(np.float32)
    bet_rep = np.repeat(beta.reshape(L, 1, HID), 128, axis=1).astype(np.float32)
    bcv_rep = np.repeat(b_conv.reshape(L, 1, HID), 128, axis=1).astype(np.float32)
    bc_rep = np.repeat(b_c.reshape(1, NCLS), G, axis=0).astype(np.float32)
    return (Win_b, Wla, Wr.astype(BF), att04, gam_rep, bet_rep, bcv_rep,
            Wc.astype(BF), bc_rep)


def make_in_maps(inputs):
    x = np.asarray(inputs["x"], dtype=np.float32)
    edge_index = np.asarray(inputs["edge_index"])
    batch = np.asarray(inputs["batch"])
    deg = np.asarray(inputs["deg"])
    Win = np.asarray(inputs["Win"], dtype=np.float32)
    b_in = np.asarray(inputs["b_in"], dtype=np.float32)
    deg_emb = np.asarray(inputs["deg_emb"], dtype=np.float32)
    Wl = np.asarray(inputs["Wl"], dtype=np.float32)
    Wr = np.asarray(inputs["Wr"], dtype=np.float32)
    att = np.asarray(inputs["att"], dtype=np.float32)
    b_conv = np.asarray(inputs["b_conv"], dtype=np.float32)
    gamma = np.asarray(inputs["gamma"], dtype=np.float32)
    beta = np.asarray(inputs["beta"], dtype=np.float32)
    Wc = np.asarray(inputs["Wc"], dtype=np.float32)
    b_c = np.asarray(inputs["b_c"], dtype=np.float32)

    key, idx_res, masks, xT, dege, pool_oh, inv_cnt = _host_prep(
        edge_index, batch, deg, deg_emb, x, b_in)
    Win_b, Wla, Wr_b, att04, gam_rep, bet_rep, bcv_rep, Wc_b, bc_rep = _prep_weights(
        Win, Wl, Wr, att, b_conv, gamma, beta, Wc, b_c)

    in_maps = []
    for c in range(NCORES):
        in_maps.append({
            "xT": xT[c], "dege": dege[c], "idx": idx_res[c], "masks": masks[c],
            "Win": Win_b, "Wla": Wla, "Wr": Wr_b, "att04": att04,
            "gamr": gam_rep, "betr": bet_rep, "bcvr": bcv_rep,
            "poh": pool_oh[c], "icnt": inv_cnt, "Wc": Wc_b, "bcr": bc_rep,
        })
    return key, in_maps


def kernel(**inputs):
    from concourse import bass_utils
    key, in_maps = make_in_maps(inputs)
    if key not in _CACHE:
        _CACHE[key] = _build(key)
    nc = _CACHE[key]
    res = bass_utils.run_bass_kernel_spmd(nc, in_maps, core_ids=list(range(NCORES)))
    kernel.last_results = res
    return res.results[0]["out"].astype(np.float32)
